# revision 9
# baseline (speedup 1.0000x reference)
"""MoCo loss kernel for Trainium2 (8 NeuronCores, Bass/Tile).

Math summary (V=2, N=1024, D=128, K=65536; all inputs L2-normalized):
  loss1 = mean_x mean_i ||q[x,i] - k[1-x,i]||^2 = 2 - (<q0,k1>_F + <q1,k0>_F)/N
  loss2 = mean_x mean_i sum_j v_ij w_ij,  w = softmax(v),  v_ij = -s_ij = 2*d_ij - 2
    where d = concat(q0 @ queue, off-diag q[x] @ q[x]^T) per row (queue part
    memoized from view 0 in the reference, replicated here).
  Per row only two sufficient statistics are needed:
    A = sum_j e^{2 d_ij}           (e^{-2} scale cancels in the ratio)
    C = sum_j (d_ij - 1) e^{2 d_ij}
  row value = B/A_true = 2*C/A.  Diagonal j==i contributes e^2 to A and 0 to C.

Sharding: queue columns are split across the 8 cores (memory-heavy tensor read
once chip-wide); intra-batch columns are split 128 per core.  Each core emits
per-row partial (A, C) accumulators; the host merges them (plain sums — no
max-subtraction needed since d in [-1, 1]) and reduces to the two scalars.

Device pipeline per 1024-column chunk (4 PSUM slots, fully overlapped):
  PE:  2 matmuls (bf16) -> d in PSUM
  ACT: E = exp(2d) -> SBUF, accum A
  DVE: P = (d-1)*E, accum C
"""

import numpy as np
import ml_dtypes

import concourse.bass as bass
import concourse.tile as tile
from concourse import mybir, bacc
from concourse.bass_utils import run_bass_kernel_spmd

V, N, D, K = 2, 1024, 128, 65536
NCORES = 8
KC = K // NCORES          # 8192 queue columns per core
CH = 1024                 # free-dim chunk per PSUM tile (2 banks)
NCH = KC // CH            # 8 chunks per i-tile
NT = N // 128             # 8 row tiles
BLK = N // NCORES         # 128 intra columns per core
NB = NT * V               # 16 intra blocks
# output column layout in the single fused output tensor
OC_AQ = 0                 # [0, 64)    A_q, col = it*NCH + ch
OC_CQ = NT * NCH          # [64, 128)  C_q
OC_AI = 2 * NT * NCH      # [128, 144) A_i, col = OC_AI + it*V + x
OC_CI = OC_AI + NB        # [144, 160) C_i
OC_FR = OC_CI + NB        # [160]      fro
OUTC = OC_FR + 1

_F32 = mybir.dt.float32
_BF16 = mybir.dt.bfloat16

_CACHE = {}


def _build():
    nc = bacc.Bacc("TRN2", target_bir_lowering=False, debug=False)

    # fused small-input tensors: one DMA each
    # small_bf cols: [q0T (N) | q1T (N) | qblk0 (BLK) | qblk1 (BLK)]
    small_bf = nc.dram_tensor("small_bf", [D, 2 * N + 2 * BLK], _BF16,
                              kind="ExternalInput")
    # qkf cols: [q0T | q1T | k1T | k0T] fp32 (loss1 pairs q[x] with k[1-x])
    qkf = nc.dram_tensor("qkf", [D, 2 * V * N], _F32, kind="ExternalInput")
    qq = nc.dram_tensor("qq", [D, KC], _BF16, kind="ExternalInput")
    outs = nc.dram_tensor("outs", [128, OUTC], _F32, kind="ExternalOutput")

    Exp = mybir.ActivationFunctionType.Exp
    sub = mybir.AluOpType.subtract
    mult = mybir.AluOpType.mult
    add = mybir.AluOpType.add
    AxX = mybir.AxisListType.X

    with tile.TileContext(nc) as tc:
        with (
            tc.tile_pool(name="singles", bufs=1) as singles,
            tc.tile_pool(name="psum", bufs=4, space="PSUM") as psum,
            tc.tile_pool(name="epool", bufs=6) as epool,
            tc.tile_pool(name="ppool", bufs=6) as ppool,
        ):
            small_sb = singles.tile([D, 2 * N + 2 * BLK], _BF16)
            nc.sync.dma_start(small_sb[:], small_bf.ap()[:])
            qkf_sb = singles.tile([D, 2 * V * N], _F32)
            nc.sync.dma_start(qkf_sb[:], qkf.ap()[:])
            # queue slice: parallel DMAs dispatched from the idle GpSimd queue
            qq_sb = singles.tile([D, KC], _BF16)
            for h in range(8):
                sl = slice(h * (KC // 8), (h + 1) * (KC // 8))
                nc.gpsimd.dma_start(qq_sb[:, sl], qq.ap()[:, sl])

            q0T_sb = small_sb[:, 0:N]
            q1T_sb = small_sb[:, N : 2 * N]
            qblk0_sb = small_sb[:, 2 * N : 2 * N + BLK]
            qblk1_sb = small_sb[:, 2 * N + BLK : 2 * N + 2 * BLK]

            out_sb = singles.tile([128, OUTC], _F32)

            qT_view = (q0T_sb, q1T_sb)
            blk_view = (qblk0_sb, qblk1_sb)

            # ---- intra-batch blocks, processed up front while qq streams in.
            # (it, x) block b = it*V + x lives at cols (b % 8)*BLK of ps2a/ps2b.
            ps2a = psum.tile([128, 8 * BLK], _F32, tag="ps")
            ps2b = psum.tile([128, 8 * BLK], _F32, tag="ps")
            ps2 = (ps2a, ps2b)
            for b in range(NB):
                it, x = divmod(b, V)
                nc.tensor.matmul(
                    ps2[b // 8][:, (b % 8) * BLK : (b % 8 + 1) * BLK],
                    qT_view[x][:, it * 128 : (it + 1) * 128],
                    blk_view[x][:],
                    start=True,
                    stop=True,
                )
            e_i = []
            for h in range(2):
                Ei = epool.tile([128, 8 * BLK], _F32, tag="E")
                nc.scalar.activation(Ei[:], ps2[h][:], Exp, bias=0.0, scale=2.0)
                e_i.append(Ei)
            p_i = []
            for h in range(2):
                Pi = ppool.tile([128, 8 * BLK], _F32, tag="Pi")
                nc.vector.scalar_tensor_tensor(
                    out=Pi[:], in0=ps2[h][:], scalar=1.0, in1=e_i[h][:],
                    op0=sub, op1=mult,
                )
                p_i.append(Pi)
            for h in range(2):
                nc.vector.tensor_reduce(
                    out=out_sb[:, OC_AI + 8 * h : OC_AI + 8 * (h + 1)],
                    in_=e_i[h][:].rearrange("p (b j) -> p b j", j=BLK),
                    axis=AxX, op=add,
                )
                nc.vector.tensor_reduce(
                    out=out_sb[:, OC_CI + 8 * h : OC_CI + 8 * (h + 1)],
                    in_=p_i[h][:].rearrange("p (b j) -> p b j", j=BLK),
                    axis=AxX, op=add,
                )

            # ---- loss1 Frobenius accumulation
            scr = ppool.tile([128, V * N], _BF16, tag="scr")
            nc.vector.scalar_tensor_tensor(
                out=scr[:], in0=qkf_sb[:, 0 : V * N], scalar=1.0,
                in1=qkf_sb[:, V * N : 2 * V * N], op0=mult, op1=mult,
                accum_out=out_sb[:, OC_FR : OC_FR + 1],
            )

            # ---- main queue loop
            for it in range(NT):
                lhs = q0T_sb[:, it * 128 : (it + 1) * 128]
                for ch in range(NCH):
                    ps = psum.tile([128, CH], _F32, tag="ps")
                    for h in range(CH // 512):
                        c0 = ch * CH + h * 512
                        nc.tensor.matmul(
                            ps[:, h * 512 : (h + 1) * 512],
                            lhs,
                            qq_sb[:, c0 : c0 + 512],
                            start=True,
                            stop=True,
                        )
                    col = it * NCH + ch
                    E = epool.tile([128, CH], _F32, tag="E")
                    nc.scalar.activation(
                        E[:], ps[:], Exp, bias=0.0, scale=2.0,
                        accum_out=out_sb[:, OC_AQ + col : OC_AQ + col + 1],
                    )
                    P = ppool.tile([128, CH], _BF16, tag="P")
                    nc.vector.scalar_tensor_tensor(
                        out=P[:], in0=ps[:], scalar=1.0, in1=E[:],
                        op0=sub, op1=mult,
                        accum_out=out_sb[:, OC_CQ + col : OC_CQ + col + 1],
                    )

            nc.sync.dma_start(outs.ap()[:], out_sb[:])

    nc.compile()
    return nc


def _get_nc():
    if "nc" not in _CACHE:
        _CACHE["nc"] = _build()
    return _CACHE["nc"]


def prepare_in_maps(q, k, queue):
    q = np.asarray(q, np.float32)
    k = np.asarray(k, np.float32)
    queue = np.asarray(queue, np.float32)

    q0T = np.ascontiguousarray(q[0].T)
    q1T = np.ascontiguousarray(q[1].T)
    q0Tb = q0T.astype(ml_dtypes.bfloat16)
    q1Tb = q1T.astype(ml_dtypes.bfloat16)
    queueb = queue.astype(ml_dtypes.bfloat16)
    qkf = np.concatenate(
        [q0T, q1T, np.ascontiguousarray(k[1].T), np.ascontiguousarray(k[0].T)],
        axis=1,
    )

    in_maps = []
    for c in range(NCORES):
        small = np.concatenate(
            [q0Tb, q1Tb,
             q0Tb[:, c * BLK : (c + 1) * BLK],
             q1Tb[:, c * BLK : (c + 1) * BLK]],
            axis=1,
        )
        in_maps.append(
            {
                "small_bf": small,
                "qkf": qkf,
                "qq": np.ascontiguousarray(queueb[:, c * KC : (c + 1) * KC]),
            }
        )

    return in_maps


def kernel(q, k, queue, **_unused):
    in_maps = prepare_in_maps(q, k, queue)
    res = run_bass_kernel_spmd(_get_nc(), in_maps, list(range(NCORES)))

    A_K = np.zeros(N, np.float64)
    C_K = np.zeros(N, np.float64)
    A_I = np.zeros((V, N), np.float64)
    C_I = np.zeros((V, N), np.float64)
    for r in res.results:
        o = r["outs"].astype(np.float64)
        # col = it*NCH + ch; row i = it*128 + p
        A_K += o[:, OC_AQ : OC_AQ + NT * NCH].reshape(128, NT, NCH).sum(2).T.reshape(N)
        C_K += o[:, OC_CQ : OC_CQ + NT * NCH].reshape(128, NT, NCH).sum(2).T.reshape(N)
        ai = o[:, OC_AI : OC_AI + NB].reshape(128, NT, V)
        ci = o[:, OC_CI : OC_CI + NB].reshape(128, NT, V)
        for x in range(V):
            A_I[x] += ai[:, :, x].T.reshape(N)
            C_I[x] += ci[:, :, x].T.reshape(N)
    A_I -= np.exp(2.0)  # remove the j == i diagonal term ((d-1)e^{2d} there is 0)

    loss2 = 0.0
    for x in range(V):
        A = A_K + A_I[x]
        C = C_K + C_I[x]
        loss2 += np.mean(2.0 * C / A)
    loss2 /= V

    fro_total = float(res.results[0]["outs"][:, OC_FR].astype(np.float64).sum())
    loss1 = 2.0 - fro_total / N

    return (np.float32(loss1), np.float32(loss2))
